# revision 30
# baseline (speedup 1.0000x reference)
"""CommNet GNN message-passing kernel for 8 Trainium2 NeuronCores.

Computation (matches the jax reference):
    h = relu(x @ enc_w1 + enc_b1) @ enc_w2 + enc_b2
    for r in range(R):
        msg[i] = mean over edges (src==i) of h[dst]
        h = h + relu(msg @ comm_w[r] + comm_b[r])
    out = relu(h @ dec_w1 + dec_b1) @ dec_w2 + dec_b2

Sharding: nodes (and their outgoing edges, partitioned by source node id)
are sharded across the 8 cores; MLP weights replicated. Each comm round
the per-core h shards are AllGather'd (bf16) into a full copy of h in
each core's DRAM; per-edge h[dst] reads are serviced by dma_gather
(256B bf16 rows).

The kernel is SWDGE-bound: descriptor generation for the gathers costs
~8ns/edge on the GpSimd Q7 pair and dominates everything else. This
version therefore:
  - consolidates gathers into per-group instructions (3 tiles/group) to
    amortize the ~1.2us/instruction fixed cost;
  - keeps h_full in bf16 (halves gather DMA + AllGather bytes);
  - splits each AllGather into 3 chunked collectives (32/12/5 tiles,
    each its own Shared tensor since Shared DRAM allows one writer) that
    are issued as soon as their tiles' h rows are written.  Chunk 0 and
    1 land mid-round; only tiny chunk 2's mesh trails the round, and the
    next round's frontloaded chunk-0/1 gathers cover it;
  - packs tile gather sections unaligned (boundary 128-blocks shared
    between neighbouring tiles, disambiguated by -1 entries in srcv);
  - builds the segment-sum one-hot in fp32 on DVE (a bf16 DVE output
    engages 2-port SBUF mode and stalls the Q7 descriptor generation),
    casting to bf16 on the idle Scalar engine for the bf16 matmul.

Per-core compute layout ("T layout": features on partitions, nodes on the
free axis) so every MLP matmul chains without transposes; segment-mean
via one-hot matmul into PSUM.
"""

import numpy as np

N = 50000
D = 128
R = 2
NCORES = 8
TILES = 49                # src-node tiles of 128 per core
NL = TILES * 128          # 6272 nodes per core
NP = NCORES * NL          # 50176 padded node count
CHUNK_TILES = (32, 12, 5)  # AllGather chunking; each chunk < 32768 rows
NSEC = len(CHUNK_TILES)
GROUP = 3                 # tiles per consolidated gather

_PROGRAM_CACHE: dict = {}

TRACE = False
LAST_RESULTS = None


def _chunks():
    """[(chunk_start_tile, n_tiles, row_offset_in_h_full), ...]"""
    out = []
    t0 = 0
    row = 0
    for ct in CHUNK_TILES:
        out.append((t0, ct, row))
        t0 += ct
        row += NCORES * ct * D
    assert t0 == TILES and row == NP
    return out


def _groups():
    """Gather groups (lists of tile ids) in processing order (chunk 0
    first; later chunks' collectives then fire before the round ends)."""
    gs = []
    for (t0, ct, _row) in _chunks():
        t = t0
        while t < t0 + ct:
            g = list(range(t, min(t + GROUP, t0 + ct)))
            gs.append(g)
            t += GROUP
    return gs


# ----------------------------------------------------------------------------
# Device program
# ----------------------------------------------------------------------------

def build_program(layout):
    """Build the SPMD Bass program from the host-side layout dict."""
    import concourse.bass as bass
    import concourse.bacc as bacc
    import concourse.mybir as mybir
    import concourse.tile as tile

    dt = mybir.dt
    d = D
    nl = NL
    n_cores = NCORES
    tiles = TILES
    n_rounds = R

    groups = layout["groups"]      # list of tile lists
    ba = layout["ba"]              # [sec][tile]: first block in group section
    bb = layout["bb"]              # [sec][tile]: end block
    g_nb = layout["g_nb"]          # [sec][group]: total section blocks
    g_col = layout["g_col"]        # [sec][group]: col offset into idx dram
    srcv_col = layout["srcv_col"]  # per tile: col offset into SRCV
    sum_b = layout["sum_b"]
    idx_cols = layout["idx_cols"]  # [sec]: total idx dram cols
    B_t = [sum(bb[c][t] - ba[c][t] for c in range(NSEC))
           for t in range(tiles)]
    gmax_nb = max(sum(g_nb[c][g] for c in range(NSEC))
                  for g in range(len(groups)))
    bmax = max(B_t)
    crows = [n_cores * ct * d for ct in CHUNK_TILES]

    nc = bacc.Bacc("TRN2", target_bir_lowering=False, debug=False,
                   num_devices=n_cores)

    # -------- kernel I/O --------
    xT_dram = nc.dram_tensor("xT", [d, nl], dt.float32, kind="ExternalInput")
    idx_drams = [nc.dram_tensor(f"idx{c}", [128, idx_cols[c]], dt.int16,
                                kind="ExternalInput") for c in range(NSEC)]
    srcv_dram = nc.dram_tensor("srcv", [128, sum_b], dt.float32,
                               kind="ExternalInput")
    winv_dram = nc.dram_tensor("winv", [d, nl], dt.float32, kind="ExternalInput")
    iota_dram = nc.dram_tensor("iota", [d, d], dt.float32, kind="ExternalInput")
    ident_dram = nc.dram_tensor("ident", [d, d], dt.float32, kind="ExternalInput")
    w_drams = {}
    for wname in ("enc_w1", "enc_w2", "dec_w1", "dec_w2"):
        w_drams[wname] = nc.dram_tensor(wname, [d, d], dt.float32,
                                        kind="ExternalInput")
    for bname in ("enc_b1", "enc_b2", "dec_b1", "dec_b2"):
        w_drams[bname] = nc.dram_tensor(bname, [d, 1], dt.float32,
                                        kind="ExternalInput")
    for r in range(n_rounds):
        w_drams[f"cw{r}"] = nc.dram_tensor(f"cw{r}", [d, d], dt.float32,
                                           kind="ExternalInput")
        w_drams[f"cb{r}"] = nc.dram_tensor(f"cb{r}", [d, 1], dt.float32,
                                           kind="ExternalInput")
    outT_dram = nc.dram_tensor("outT", [d, nl], dt.float32,
                               kind="ExternalOutput")

    Relu = mybir.ActivationFunctionType.Relu
    Ident = mybir.ActivationFunctionType.Identity
    EQ = mybir.AluOpType.is_equal
    MUL = mybir.AluOpType.mult
    ADD = mybir.AluOpType.add

    with tile.TileContext(nc) as tc:
        with (
            tc.tile_pool(name="persist", bufs=1) as pp,
            tc.tile_pool(name="work", bufs=3) as wp,
            tc.tile_pool(name="gather", bufs=2) as gp,
            tc.tile_pool(name="ohpool", bufs=2) as op_,
            tc.tile_pool(name="ohfpool", bufs=1) as ofp,
            tc.tile_pool(name="psum", bufs=2, space="PSUM") as ps,
            tc.tile_pool(name="psum2", bufs=2, space="PSUM") as ps2,
            tc.tile_pool(name="dram", bufs=1, space="DRAM") as dp,
        ):
            # ---- persistent SBUF state ----
            xT = pp.tile([d, nl], dt.float32)
            hT = pp.tile([d, nl], dt.float32)
            winv = pp.tile([d, nl], dt.float32)
            iota = pp.tile([d, d], dt.float32)
            ident = pp.tile([d, d], dt.float32)
            wt = {}
            for wname in ("enc_w1", "enc_w2", "dec_w1", "dec_w2"):
                wt[wname] = pp.tile([d, d], dt.float32, tag=wname, name=wname)
            for bname in ("enc_b1", "enc_b2", "dec_b1", "dec_b2"):
                wt[bname] = pp.tile([d, 1], dt.float32, tag=bname, name=bname)
            for r in range(n_rounds):
                wt[f"cw{r}"] = pp.tile([d, d], dt.float32, tag=f"cw{r}",
                                       name=f"cw{r}")
                wt[f"cb{r}"] = pp.tile([d, 1], dt.float32, tag=f"cb{r}",
                                       name=f"cb{r}")

            nc.sync.dma_start(xT[:], xT_dram[:])
            nc.sync.dma_start(winv[:], winv_dram[:])
            nc.sync.dma_start(iota[:], iota_dram[:])
            nc.sync.dma_start(ident[:], ident_dram[:])
            for k, t in wt.items():
                nc.sync.dma_start(t[:], w_drams[k][:])

            # ---- DRAM scratch (bf16 h distribution) ----
            # One Shared tensor per AllGather chunk (single-writer rule).
            ag_in = dp.tile([nl, d], dt.bfloat16)
            h_sec = [[dp.tile([crows[c], d], dt.bfloat16, addr_space="Shared",
                              tag=f"h{r}_{c}", name=f"h{r}_{c}")
                      for c in range(NSEC)] for r in range(n_rounds)]

            def write_h_rows(t):
                """transpose hT[:, tile t] -> bf16 [node, feat] rows -> ag_in."""
                tsl = slice(t * d, (t + 1) * d)
                psT = ps2.tile([d, d], dt.float32, tag="psT")
                nc.tensor.transpose(psT[:], hT[:, tsl], ident[:])
                rowt = wp.tile([d, d], dt.bfloat16, tag="rowt")
                nc.scalar.copy(rowt[:], psT[:])
                nc.sync.dma_start(ag_in[tsl, :], rowt[:])

            def issue_ag(r, c):
                """AllGather chunk c of round r's h."""
                t0, ct, _row = _chunks()[c]
                nc.gpsimd.collective_compute(
                    "AllGather",
                    mybir.AluOpType.bypass,
                    replica_groups=[list(range(n_cores))],
                    ins=[ag_in[t0 * d:(t0 + ct) * d, :].opt()],
                    outs=[h_sec[r][c].opt()],
                )

            # ================= encoder, chunked ======
            for c, (t0, ct, _row) in enumerate(_chunks()):
                t = t0
                while t < t0 + ct:
                    eg = min(4, t0 + ct - t)
                    gsl = slice(t * d, (t + eg) * d)
                    psA = ps.tile([d, 4 * d], dt.float32, tag="psA",
                                  padded_shape=[d, 4 * d])
                    nc.tensor.matmul(psA[:, 0:eg * d], wt["enc_w1"][:],
                                     xT[:, gsl], start=True, stop=True)
                    h1T = wp.tile([d, 4 * d], dt.float32, tag="h1T")
                    nc.scalar.activation(h1T[:, 0:eg * d], psA[:, 0:eg * d],
                                         Relu, bias=wt["enc_b1"][:])
                    psB = ps.tile([d, 4 * d], dt.float32, tag="psB",
                                  padded_shape=[d, 4 * d])
                    nc.tensor.matmul(psB[:, 0:eg * d], wt["enc_w2"][:],
                                     h1T[:, 0:eg * d], start=True, stop=True)
                    nc.scalar.activation(hT[:, gsl], psB[:, 0:eg * d], Ident,
                                         bias=wt["enc_b2"][:])
                    for tt in range(t, t + eg):
                        write_h_rows(tt)
                    t += eg
                issue_ag(0, c)

            # ================= comm rounds =================
            for r in range(n_rounds):
                srcs = [h_sec[r][c][:] for c in range(NSEC)]
                chunk_done = {t0 + ct - 1: c for c, (t0, ct, _row)
                              in enumerate(_chunks())}

                gbufs = {}

                def alloc_gbuf(g):
                    gbufs[g] = gp.tile([128, gmax_nb, d], dt.bfloat16,
                                       tag="gbuf", name=f"gbuf_r{r}g{g}")

                def emit_sec(g, c):
                    """Gather group g's section-c slots."""
                    nb = g_nb[c][g]
                    if nb == 0:
                        return
                    off = sum(g_nb[cc][g] for cc in range(c))
                    n_i = nb * 128
                    it = wp.tile([128, (gmax_nb * 128) // 16], dt.int16,
                                 tag=f"idx{c}", name=f"idx_r{r}g{g}c{c}")
                    nc.sync.dma_start(
                        it[:, 0:n_i // 16],
                        idx_drams[c][:, g_col[c][g]:g_col[c][g] + n_i // 16])
                    nc.gpsimd.dma_gather(gbufs[g][:, off:off + nb, :],
                                         srcs[c], it[:, 0:n_i // 16],
                                         n_i, n_i, d, single_packet=False)

                # Frontload groups 0/1's chunk-0 and chunk-1 gathers (those
                # collectives landed mid-previous-round); chunk 2's tiny
                # mesh then hides under them.
                ng = len(groups)
                for g in range(min(2, ng)):
                    alloc_gbuf(g)
                for c in range(NSEC):
                    for g in range(min(2, ng)):
                        emit_sec(g, c)
                for g, gtiles in enumerate(groups):
                    gbuf = gbufs[g]

                    for t in gtiles:
                        tsl = slice(t * d, (t + 1) * d)
                        bt = B_t[t]
                        # one-hot [128, bt*d]: oh[p, b*d + j] = (srcv[p,b] == j)
                        srcv = wp.tile([128, bmax], dt.float32, tag="srcv")
                        nc.sync.dma_start(
                            srcv[:, 0:bt],
                            srcv_dram[:, srcv_col[t]:srcv_col[t] + bt])
                        # DVE builds the one-hot in fp32 (a bf16 DVE output
                        # would engage 2-port mode and contend with the Q7's
                        # SBUF descriptor rings, stalling the gathers); the
                        # idle Scalar engine casts to bf16 for the matmul.
                        ohf = ofp.tile([128, bmax * d], dt.float32, tag="ohf")
                        oh = op_.tile([128, bmax * d], dt.bfloat16, tag="oh")
                        in0 = bass.AP(srcv.tensor, srcv.offset,
                                      [srcv.ap[0], [1, bt], [0, d]])
                        in1 = bass.AP(iota.tensor, iota.offset,
                                      [iota.ap[0], [0, bt], [1, d]])
                        out_oh = bass.AP(ohf.tensor, ohf.offset,
                                         [ohf.ap[0], [d, bt], [1, d]])
                        nc.vector.tensor_tensor(out_oh, in0, in1, EQ)
                        nc.scalar.copy(oh[:, 0:bt * d], ohf[:, 0:bt * d])
                        # segment sums: psM[f, n] += gathered_b.T @ onehot_b
                        psM = ps.tile([d, d], dt.float32, tag="psA")
                        nblk = 0
                        for c in range(NSEC):
                            boff = sum(g_nb[cc][g] for cc in range(c))
                            for b in range(ba[c][t], bb[c][t]):
                                nc.tensor.matmul(
                                    psM[:], gbuf[:, boff + b, :],
                                    oh[:, nblk * d:(nblk + 1) * d],
                                    start=(nblk == 0), stop=(nblk == bt - 1))
                                nblk += 1
                        # mean + comm MLP + residual
                        msgT = wp.tile([d, d], dt.float32, tag="msgT")
                        nc.vector.tensor_tensor(msgT[:], psM[:],
                                                winv[:, tsl], MUL)
                        psU = ps.tile([d, d], dt.float32, tag="psB")
                        nc.tensor.matmul(psU[:], wt[f"cw{r}"][:], msgT[:],
                                         start=True, stop=True)
                        updT = wp.tile([d, d], dt.float32, tag="updT")
                        nc.scalar.activation(updT[:], psU[:], Relu,
                                             bias=wt[f"cb{r}"][:])
                        nc.vector.tensor_tensor(hT[:, tsl], hT[:, tsl],
                                                updT[:], ADD)
                        if r + 1 < n_rounds:
                            write_h_rows(t)
                            if t in chunk_done:
                                issue_ag(r + 1, chunk_done[t])
                        else:
                            # final round: decoder fused per tile
                            psD = ps.tile([d, d], dt.float32, tag="psA",
                                          name="psD")
                            nc.tensor.matmul(psD[:], wt["dec_w1"][:],
                                             hT[:, tsl], start=True, stop=True)
                            d1T = wp.tile([d, d], dt.float32, tag="d1T")
                            nc.scalar.activation(d1T[:], psD[:], Relu,
                                                 bias=wt["dec_b1"][:])
                            psE = ps.tile([d, d], dt.float32, tag="psB",
                                          name="psE")
                            nc.tensor.matmul(psE[:], wt["dec_w2"][:], d1T[:],
                                             start=True, stop=True)
                            oT = wp.tile([d, d], dt.float32, tag="oT")
                            nc.scalar.activation(oT[:], psE[:], Ident,
                                                 bias=wt["dec_b2"][:])
                            nc.sync.dma_start(outT_dram[:, tsl], oT[:])

                    if g + 2 < ng:
                        alloc_gbuf(g + 2)
                        for c in range(NSEC):
                            emit_sec(g + 2, c)

    nc.compile()
    return nc


# ----------------------------------------------------------------------------
# Host-side preparation
# ----------------------------------------------------------------------------

def _wrap_idx(idx):
    """int16 idx vector -> [128, n/16] layout: pos j -> (j%16, j//16), x8."""
    n = len(idx)
    a = np.zeros((16, n // 16), np.int16)
    a[np.arange(n) % 16, np.arange(n) // 16] = idx
    return np.tile(a, (8, 1))


def _row_remap():
    """node id (0..NP-1, old layout core-major) -> h row (chunk-major)."""
    remap = np.empty(NP, np.int64)
    n = np.arange(NP)
    k = n // NL
    t = (n % NL) // D
    p = n % D
    for (t0, ct, row) in _chunks():
        m = (t >= t0) & (t < t0 + ct)
        remap[n[m]] = (row + k[m] * ct * D + (t[m] - t0) * D + p[m])
    return remap


def host_prep(x, edge_index):
    """Shard + pad inputs; build per-core gather/one-hot side data."""
    d = D
    nl = NL
    n_real = x.shape[0]

    src = np.asarray(edge_index[0]).astype(np.int64)
    dst = np.asarray(edge_index[1]).astype(np.int64)

    cnt = np.bincount(src, minlength=NP).astype(np.float32)
    winv_full = 1.0 / np.maximum(cnt, 1.0)

    x_pad = np.zeros((NP, d), np.float32)
    x_pad[:n_real] = np.asarray(x, np.float32)

    remap = _row_remap()
    dstm = remap[dst]                # h row of each edge's dst

    # chunk row boundaries
    row_off = [c[2] for c in _chunks()] + [NP]

    # sort edges once by (tile, dst-row)
    tile_of_edge = src // d
    order = np.lexsort((dstm, tile_of_edge))
    src_s, dstm_s = src[order], dstm[order]
    tile_s = tile_of_edge[order]
    sec_s = np.searchsorted(row_off, dstm_s, side="right") - 1

    n_tiles_g = NCORES * TILES
    tile_start = np.searchsorted(tile_s, np.arange(n_tiles_g))
    tile_end = np.searchsorted(tile_s, np.arange(n_tiles_g) + 1)
    # per (core-tile, section) counts
    cnt_cs = np.zeros((n_tiles_g, NSEC), np.int64)
    for g in range(n_tiles_g):
        s0, s1 = tile_start[g], tile_end[g]
        cnt_cs[g] = np.bincount(sec_s[s0:s1], minlength=NSEC)

    core_ix = np.arange(NCORES) * TILES
    # cross-core per-tile maxima (SPMD immediates)
    m_sec = [[max(1, int(cnt_cs[core_ix + t, c].max())) for t in range(TILES)]
             for c in range(NSEC)]

    groups = _groups()
    # unaligned slot packing per section
    sec_start = [[0] * TILES for _ in range(NSEC)]
    g_nb = [[0] * len(groups) for _ in range(NSEC)]
    g_col = [[0] * len(groups) for _ in range(NSEC)]
    for c in range(NSEC):
        col = 0
        for g, gtiles in enumerate(groups):
            cc = 0
            for t in gtiles:
                sec_start[c][t] = cc
                cc += m_sec[c][t]
            g_nb[c][g] = (cc + 127) // 128
            g_col[c][g] = col
            col += g_nb[c][g] * 8

    ba = [[sec_start[c][t] // 128 for t in range(TILES)] for c in range(NSEC)]
    bb = [[(sec_start[c][t] + m_sec[c][t] + 127) // 128 for t in range(TILES)]
          for c in range(NSEC)]

    srcv_col = [0] * TILES
    sb = 0
    for t in range(TILES):
        srcv_col[t] = sb
        sb += sum(bb[c][t] - ba[c][t] for c in range(NSEC))

    layout = {
        "groups": groups, "ba": ba, "bb": bb,
        "g_nb": g_nb, "g_col": g_col, "srcv_col": srcv_col,
        "sum_b": sb,
        "idx_cols": [sum(g_nb[c]) * 8 for c in range(NSEC)],
        "m_sec": m_sec,
    }

    per_core = []
    for k in range(NCORES):
        idx_all = [np.zeros((128, layout["idx_cols"][c]), np.int16)
                   for c in range(NSEC)]
        srcv_all = np.full((128, sb), -1.0, np.float32)
        for g, gtiles in enumerate(groups):
            idx_g = [np.zeros(g_nb[c][g] * 128, np.int16)
                     for c in range(NSEC)]
            for t in gtiles:
                gt = k * TILES + t
                s0, s1 = tile_start[gt], tile_end[gt]
                bt = sum(bb[c][t] - ba[c][t] for c in range(NSEC))
                slot_src = np.full(bt * 128, -1.0, np.float32)
                srel = 0
                for c in range(NSEC):
                    e_c = np.flatnonzero(sec_s[s0:s1] == c) + s0
                    o = sec_start[c][t]
                    idx_g[c][o:o + len(e_c)] = dstm_s[e_c] - row_off[c]
                    rel = srel + (o - ba[c][t] * 128)
                    slot_src[rel:rel + len(e_c)] = \
                        (src_s[e_c] - gt * d).astype(np.float32)
                    srel += (bb[c][t] - ba[c][t]) * 128
                srcv_all[:, srcv_col[t]:srcv_col[t] + bt] = \
                    slot_src.reshape(bt, 128).T
            for c in range(NSEC):
                idx_all[c][:, g_col[c][g]:g_col[c][g] + g_nb[c][g] * 8] = \
                    _wrap_idx(idx_g[c])
        ksl = slice(k * nl, (k + 1) * nl)
        pc = {
            "xT": np.ascontiguousarray(x_pad[ksl].T),
            "srcv": srcv_all,
            "winv": np.ascontiguousarray(
                np.tile(winv_full[ksl][None, :], (d, 1))),
        }
        for c in range(NSEC):
            pc[f"idx{c}"] = idx_all[c]
        per_core.append(pc)
    return per_core, layout


def kernel(x, edge_index, enc_w1, enc_b1, enc_w2, enc_b2,
           comm_w, comm_b, dec_w1, dec_b1, dec_w2, dec_b2):
    from concourse.bass_utils import run_bass_kernel_spmd

    x = np.asarray(x)
    n_real = x.shape[0]
    per_core, layout = host_prep(x, np.asarray(edge_index))

    key = tuple(tuple(m) for m in layout["m_sec"])
    if key not in _PROGRAM_CACHE:
        _PROGRAM_CACHE[key] = build_program(layout)
    nc = _PROGRAM_CACHE[key]

    iota_np = np.tile(np.arange(D, dtype=np.float32)[None, :], (D, 1))
    ident_np = np.eye(D, dtype=np.float32)
    shared = {
        "iota": iota_np,
        "ident": ident_np,
        "enc_w1": np.asarray(enc_w1, np.float32),
        "enc_w2": np.asarray(enc_w2, np.float32),
        "dec_w1": np.asarray(dec_w1, np.float32),
        "dec_w2": np.asarray(dec_w2, np.float32),
        "enc_b1": np.asarray(enc_b1, np.float32).reshape(D, 1),
        "enc_b2": np.asarray(enc_b2, np.float32).reshape(D, 1),
        "dec_b1": np.asarray(dec_b1, np.float32).reshape(D, 1),
        "dec_b2": np.asarray(dec_b2, np.float32).reshape(D, 1),
    }
    for r in range(R):
        shared[f"cw{r}"] = np.asarray(comm_w[r], np.float32)
        shared[f"cb{r}"] = np.asarray(comm_b[r], np.float32).reshape(D, 1)

    in_maps = [{**shared, **pc} for pc in per_core]
    res = run_bass_kernel_spmd(nc, in_maps, core_ids=list(range(NCORES)),
                               trace=TRACE)
    global LAST_RESULTS
    LAST_RESULTS = res

    out = np.empty((NCORES * NL, D), np.float32)
    for k in range(NCORES):
        out[k * NL:(k + 1) * NL] = res.results[k]["outT"].T
    return out[:n_real]
